# revision 19
# baseline (speedup 1.0000x reference)
"""Trainium2 Bass kernel for nn_AttentionBlock (B=8, C=1024, L=1024, H=16, G=32).

Data-parallel over batch: 8 samples -> 8 NeuronCores, one sample per core.

v2 design notes (vs v1): the kernel is ACT-bound -- the 16.8M softmax exps
cost ~1.03us per 1024-element ACTIVATE at 1.2GHz, ~132us total per core.
Everything else must hide under that stream while keeping the PE busy enough
to hold its 2.4GHz p-state (idle gaps reset the clock ramp to 1.2GHz):

  - x is uploaded as bf16 (2MB not 4MB): the last tile lands ~7us in.
  - GroupNorm per batch-of-4 tiles; the x*scale+bias fp8 applies are split
    into L-halves and alternate ACT/DVE so the n0 halves (which gate the
    first q/k chains) finish early.
  - Attention runs tcn-major (t-chunk outer, head-pair inner).  Window
    (j, n) = 8 scoresT matmul pairs (row-group concurrent bf16), 8 exps
    (the pacing stream), 4 deferred fp8-DR attention-value matmuls, and a
    normalization epilogue (reciprocal on DVE straight from the PSUM
    denominator row, gpsimd partition-broadcast, one multiply into a8).
  - PE filler generators are pumped between exps so the PE never waits:
    next pair's q/k DR projection (chains ordered k-n0, q-n0, k-n1, q-n1
    so the next window's early scores can start after two chains), the
    v^T projection (tcn0 windows), and the output projection for t-chunk 0
    (tcn1 windows).  Only proj n=1 (+epilogue) remains after the last exp.
  - The ACT queue carries ONLY: one table load (natural_log_exp), the two
    GN rsqrts, half the GN applies, and the 128 exps.  All DMAs ride the
    sync/vector/gpsimd queues.
  - PSUM: scores ring 2x[128,2,512] (4 banks) + ps2 2x[65,512] (2) +
    qk accumulator (1) + filler accumulator (1) = 8 banks.

Numerics as v1: fp8e4m3 DoubleRow matmuls for all K=1024 contractions,
bf16 scores, softmax exp(z/8 - 2) without max-subtraction, denominator via
an exact-1.0 column 64 in vT, v bias folded into the proj bias on host.
"""

import numpy as np
import ml_dtypes

import concourse.bass as bass
import concourse.bacc as bacc
import concourse.tile as tile
from concourse import mybir
from concourse.bass_utils import run_bass_kernel_spmd

F32 = mybir.dt.float32
BF16 = mybir.dt.bfloat16
F8 = mybir.dt.float8e4
DR = mybir.MatmulPerfMode.DoubleRow

B, C, L, H = 8, 1024, 1024, 16
GROUPS = 32
CH = C // H          # 64 per-head channels
EPS = 1e-5
NT = C // 128        # 8 channel tiles
LT = L // 512        # 2 free-dim chunks of 512
PAIRS = H // 2       # 8 head pairs
KG = 4               # DoubleRow contraction groups of 256 channels
SHIFT = 2.0          # exp(z/8 - SHIFT); cancels in softmax normalization


def declare_params(nc):
    p = {}
    p["x"] = nc.declare_dram_parameter("x", [C, L], F32, isOutput=False)
    p["q_w8"] = nc.declare_dram_parameter("q_w8", [128, PAIRS, KG, 2, 128],
                                          F8, isOutput=False)
    p["k_w8"] = nc.declare_dram_parameter("k_w8", [128, PAIRS, KG, 2, 128],
                                          F8, isOutput=False)
    p["v_w8"] = nc.declare_dram_parameter("v_w8", [128, KG, 2, C], F8,
                                          isOutput=False)
    p["p_w8"] = nc.declare_dram_parameter("p_w8", [128, KG, 2, C], F8,
                                          isOutput=False)
    # packed small consts: [ag(4) | nw(8) | nb(8) | qb(8) | kb(8) | pb(8)]
    p["cpk"] = nc.declare_dram_parameter("cpk", [128, 44], F32,
                                         isOutput=False)
    p["A2T"] = nc.declare_dram_parameter("A2T", [4, 128], F32, isOutput=False)
    p["out"] = nc.declare_dram_parameter("out", [C, L], F32, isOutput=True)
    return p


def emit(nc, tc, ctx, params, out_handle=None):
    """Emit one whole per-core computation inside an open TileContext."""
    from contextlib import ExitStack

    x_d = params["x"]
    out_d = params["out"] if out_handle is None else out_handle
    x_ap, out_ap = x_d.ap(), out_d.ap()

    # ---- persistent SBUF pools ---------------------------------------
    consts = ctx.enter_context(tc.tile_pool(name="consts", bufs=1))
    xp = ctx.enter_context(tc.tile_pool(name="xp", bufs=NT))
    xb8_p = ctx.enter_context(tc.tile_pool(name="xb8", bufs=KG))
    wq_p = ctx.enter_context(tc.tile_pool(name="wq", bufs=1))
    vT_p = ctx.enter_context(tc.tile_pool(name="vT", bufs=KG))
    a8_p = ctx.enter_context(tc.tile_pool(name="a8", bufs=KG))
    gn_p = ctx.enter_context(tc.tile_pool(name="gn", bufs=1))
    qk_p = ctx.enter_context(tc.tile_pool(name="qk", bufs=2 * PAIRS))
    exp_p = ctx.enter_context(tc.tile_pool(name="expp", bufs=4))
    rc_p = ctx.enter_context(tc.tile_pool(name="rcp", bufs=2))
    rcb_p = ctx.enter_context(tc.tile_pool(name="rcbp", bufs=2))
    out_p = ctx.enter_context(tc.tile_pool(name="outp", bufs=3))
    xr_p = ctx.enter_context(tc.tile_pool(name="xrp", bufs=3))

    # ---- DMA schedule (ACT queue carries no DMAs) --------------------
    #   gpsimd: pair-0 q/k weights (small, needed first), x1, x3, vw
    #   sync:   x0, x2, x4, x6, qw/kw rest, consts, pw
    #   vector: memsets, x5, x7, then GN compute
    qw0_sb = wq_p.tile([128, 1, KG, 2, 128], F8, tag="qw0", name="qw0")
    kw0_sb = wq_p.tile([128, 1, KG, 2, 128], F8, tag="kw0", name="kw0")
    nc.gpsimd.dma_start(out=qw0_sb, in_=params["q_w8"].ap()[:, 0:1])
    nc.gpsimd.dma_start(out=kw0_sb, in_=params["k_w8"].ap()[:, 0:1])

    onesg = consts.tile([128, 2 * H], F32, tag="onesg", name="onesg")
    nc.vector.memset(onesg, 1.0)
    eps_sb = consts.tile([4, 1], F32, tag="eps", name="eps")
    nc.vector.memset(eps_sb, EPS)
    shift_sb = consts.tile([128, 1], F32, tag="shift", name="shift")
    nc.vector.memset(shift_sb, -SHIFT)

    # packed consts first on sync (one tiny DMA, lands ~1us)
    cpk_sb = consts.tile([128, 44], F32, tag="cpk", name="cpk")
    nc.sync.dma_start(out=cpk_sb, in_=params["cpk"].ap())
    a2_sb = consts.tile([4, 128], F32, tag="a2", name="a2")
    nc.sync.dma_start(out=a2_sb, in_=params["A2T"].ap())
    ag_sb = cpk_sb[:, 0:4]
    nw_sb = cpk_sb[:, 4:12]
    nb_sb = cpk_sb[:, 12:20]
    qb_sb = cpk_sb[:, 20:28]
    kb_sb = cpk_sb[:, 28:36]
    pb_sb = cpk_sb[:, 36:44]

    # x5/x7 ride the scalar queue: issued in the first ~1.3us, long before
    # the first ACT compute (~4us); the ACT queue is clean afterwards.
    xt = []
    x_eng = [nc.sync, nc.gpsimd, nc.sync, nc.gpsimd,
             nc.sync, nc.scalar, nc.sync, nc.scalar]
    for t in range(NT):
        tt = xp.tile([128, L], F32, tag="x_t", name="x_t")
        x_eng[t].dma_start(out=tt, in_=x_ap[t * 128:(t + 1) * 128, :])
        xt.append(tt)

    qwr_sb = wq_p.tile([128, PAIRS - 1, KG, 2, 128], F8, tag="qwr",
                       name="qwr")
    kwr_sb = wq_p.tile([128, PAIRS - 1, KG, 2, 128], F8, tag="kwr",
                       name="kwr")
    nc.sync.dma_start(out=qwr_sb, in_=params["q_w8"].ap()[:, 1:PAIRS])
    nc.sync.dma_start(out=kwr_sb, in_=params["k_w8"].ap()[:, 1:PAIRS])

    vw_sb = wq_p.tile([128, KG, 2, C], F8, tag="vw", name="vw")
    nc.gpsimd.dma_start(out=vw_sb, in_=params["v_w8"].ap())
    pw_sb = wq_p.tile([128, KG, 2, C], F8, tag="pw", name="pw")
    nc.sync.dma_start(out=pw_sb, in_=params["p_w8"].ap())

    def qk_w(name, j):
        if j == 0:
            return (qw0_sb if name == "q" else kw0_sb)[:, 0]
        return (qwr_sb if name == "q" else kwr_sb)[:, j - 1]

    # vT2[scp][s, sub, h, 0:64] = v^T for s-chunk scp*2+sub; col 64 == 1.0
    vT2 = []
    for scp in range(KG):
        vt_t = vT_p.tile([128, 2, H, CH + 1], F8, tag="vT_t", name="vT_t")
        nc.vector.tensor_copy(
            out=vt_t[:, :, :, CH:CH + 1],
            in_=onesg.rearrange("p (a g o) -> p a g o", a=2, o=1))
        vT2.append(vt_t)

    xb8 = [xb8_p.tile([128, 2, L], F8, tag="xb8_t", name="xb8_t")
           for _ in range(KG)]
    a8 = [a8_p.tile([128, 2, L], F8, tag="a8_t", name="a8_t")
          for _ in range(KG)]

    # ================= GroupNorm (2 batches of 4 tiles) ===============
    scale_sb = gn_p.tile([128, NT], F32, tag="scale", name="scale")
    bias_sb = gn_p.tile([128, NT], F32, tag="bias", name="bias")
    stats6 = gn_p.tile([128, NT, 2, 6], F32, tag="st6", name="st6")
    mv_all = gn_p.tile([128, NT, 2], F32, tag="mva", name="mva")
    stats = gn_p.tile([128, 2 * NT], F32, tag="stats", name="stats")
    mv16 = gn_p.tile([4, 2 * NT], F32, tag="mv16", name="mv16")
    NB = NT // 2

    qk_tiles = {}

    with ExitStack() as head_ps:
        gn_ps = head_ps.enter_context(
            tc.tile_pool(name="gnps", bufs=2, space=bass.MemorySpace.PSUM))
        qk0_ps = head_ps.enter_context(
            tc.tile_pool(name="qk0ps", bufs=4, space=bass.MemorySpace.PSUM))

        for b in range(2):
            ts0 = b * NB
            sl = slice(ts0, ts0 + NB)
            sl2 = slice(NT + ts0, NT + ts0 + NB)
            for t in range(ts0, ts0 + NB):
                for h2 in range(2):
                    nc.vector.bn_stats(
                        out=stats6[:, t, h2, :],
                        in_=xt[t][:, h2 * 512:(h2 + 1) * 512],
                    )
                nc.vector.bn_aggr(out=mv_all[:, t, :],
                                  in_=stats6[:, t, :, :])
            # stats: [mean_c | e2_c] per channel, e2 = var + mean^2
            nc.vector.tensor_copy(out=stats[:, sl], in_=mv_all[:, sl, 0])
            nc.vector.tensor_tensor(out=stats[:, sl2], in0=mv_all[:, sl, 0],
                                    in1=mv_all[:, sl, 0],
                                    op=mybir.AluOpType.mult)
            nc.vector.tensor_tensor(out=stats[:, sl2], in0=stats[:, sl2],
                                    in1=mv_all[:, sl, 1],
                                    op=mybir.AluOpType.add)

            gps = gn_ps.tile([4, 2 * NB], F32, tag="gps", name="gps")
            nc.tensor.matmul(gps[:, 0:NB], ag_sb, stats[:, sl])
            nc.tensor.matmul(gps[:, NB:], ag_sb, stats[:, sl2])
            inv_n = 1.0 / 32
            nc.vector.tensor_scalar_mul(out=mv16[:, sl], in0=gps[:, 0:NB],
                                        scalar1=inv_n)
            e2 = gn_p.tile([4, NB], F32, tag="e2", name="e2")
            nc.vector.tensor_scalar_mul(out=e2, in0=gps[:, NB:],
                                        scalar1=inv_n)
            m2 = gn_p.tile([4, NB], F32, tag="m2", name="m2")
            nc.vector.tensor_tensor(out=m2, in0=mv16[:, sl],
                                    in1=mv16[:, sl],
                                    op=mybir.AluOpType.mult)
            var = gn_p.tile([4, NB], F32, tag="var", name="var")
            nc.vector.tensor_tensor(out=var, in0=e2, in1=m2,
                                    op=mybir.AluOpType.subtract)
            lnv = gn_p.tile([4, NB], F32, tag="lnv", name="lnv")
            nc.scalar.activation(out=lnv, in_=var,
                                 func=mybir.ActivationFunctionType.Ln,
                                 bias=eps_sb, scale=1.0)
            # istd = exp(-0.5*ln(var+eps)); Ln/Exp share one ACT table set
            nc.scalar.activation(out=mv16[:, sl2], in_=lnv,
                                 func=mybir.ActivationFunctionType.Exp,
                                 scale=-0.5)

            bc = gn_ps.tile([128, 2 * NB], F32, tag="bc", name="bc")
            nc.tensor.matmul(bc[:, 0:NB], a2_sb, mv16[:, sl])
            nc.tensor.matmul(bc[:, NB:], a2_sb, mv16[:, sl2])

            nc.vector.tensor_tensor(out=scale_sb[:, sl], in0=nw_sb[:, sl],
                                    in1=bc[:, NB:],
                                    op=mybir.AluOpType.mult)
            tmp = gn_p.tile([128, NB], F32, tag="tmp", name="tmp")
            nc.vector.tensor_tensor(out=tmp, in0=bc[:, 0:NB],
                                    in1=scale_sb[:, sl],
                                    op=mybir.AluOpType.mult)
            nc.vector.tensor_tensor(out=bias_sb[:, sl], in0=nb_sb[:, sl],
                                    in1=tmp, op=mybir.AluOpType.subtract)

            # fp8 applies on DVE, split into L-halves: n0 halves first
            # (they gate the first q/k chains).
            for half in range(2):
                for t in range(ts0, ts0 + NB):
                    nc.vector.tensor_scalar(
                        out=xb8[t // 2][:, t % 2,
                                        half * 512:(half + 1) * 512],
                        in0=xt[t][:, half * 512:(half + 1) * 512],
                        scalar1=scale_sb[:, t:t + 1],
                        scalar2=bias_sb[:, t:t + 1],
                        op0=mybir.AluOpType.mult,
                        op1=mybir.AluOpType.add)

        # ---- prelude: pair-0 q/k projection (4 parallel PSUM banks) --
        dq0 = qk_p.tile([128, L], BF16, tag="qj", name="qj")
        dk0 = qk_p.tile([128, L], BF16, tag="kj", name="kj")
        qk_tiles[0] = (dq0, dk0)
        chains = (("k", 0), ("q", 0), ("k", 1), ("q", 1))
        accs = {c: qk0_ps.tile([128, 512], F32, tag="qk0a", name="qk0a")
                for c in chains}
        for g in range(KG):
            for name, nn_ in chains:
                nc.tensor.matmul(
                    accs[(name, nn_)], qk_w(name, 0)[:, g],
                    xb8[g][:, :, nn_ * 512:(nn_ + 1) * 512],
                    start=(g == 0), stop=(g == KG - 1), perf_mode=DR)
        for name, nn_ in chains:
            dst = dq0 if name == "q" else dk0
            b_sb = qb_sb if name == "q" else kb_sb
            nc.vector.tensor_scalar_add(
                out=dst[:, nn_ * 512:(nn_ + 1) * 512],
                in0=accs[(name, nn_)], scalar1=b_sb[:, 0:1])

    # ================= attention: tcn-major windows ===================
    with ExitStack() as attn:
        m1_p = attn.enter_context(
            tc.tile_pool(name="m1p", bufs=2, space=bass.MemorySpace.PSUM))
        ps2_p = attn.enter_context(
            tc.tile_pool(name="ps2p", bufs=2, space=bass.MemorySpace.PSUM))
        qkps = attn.enter_context(
            tc.tile_pool(name="qkps", bufs=1, space=bass.MemorySpace.PSUM))
        fil_ps = attn.enter_context(
            tc.tile_pool(name="filps", bufs=1, space=bass.MemorySpace.PSUM))

        def qk_chains(j, chains):
            """q/k DR projection chains for pair j in yield-sized chunks."""
            for name, nn_ in chains:
                acc = qkps.tile([128, 512], F32, tag="qka", name="qka")
                for g in range(KG):
                    nc.tensor.matmul(
                        acc, qk_w(name, j)[:, g],
                        xb8[g][:, :, nn_ * 512:(nn_ + 1) * 512],
                        start=(g == 0), stop=(g == KG - 1), perf_mode=DR)
                    yield
                dst = qk_tiles[j][0 if name == "q" else 1]
                b_sb = qb_sb if name == "q" else kb_sb
                nc.vector.tensor_scalar_add(
                    out=dst[:, nn_ * 512:(nn_ + 1) * 512],
                    in0=acc, scalar1=b_sb[:, j:j + 1])

        def qk_gen_a(j):
            """k-n0, q-n0, k-n1 for pair j (everything its tcn0 window
            touches; q-n1 is deferred to a tcn1 filler)."""
            qk_tiles[j] = (qk_p.tile([128, L], BF16, tag="qj", name="qj"),
                           qk_p.tile([128, L], BF16, tag="kj", name="kj"))
            yield from qk_chains(j, (("k", 0), ("q", 0), ("k", 1)))

        def qk_gen_b(j):
            yield from qk_chains(j, (("q", 1),))

        def vt_chain(lc, half):
            acc = fil_ps.tile([128, 512], F32, tag="vac", name="vac")
            for g in range(KG):
                nc.tensor.matmul(
                    acc, xb8[g][:, :, lc * 128:(lc + 1) * 128],
                    vw_sb[:, g, :, half * 512:(half + 1) * 512],
                    start=(g == 0), stop=(g == KG - 1), perf_mode=DR)
                yield
            nc.vector.tensor_copy(
                out=vT2[lc // 2][:, lc % 2, 8 * half:8 * half + 8, 0:CH],
                in_=acc.rearrange("p (h c) -> p h c", c=CH))

        def vt_gen(first_done):
            # half 0 = heads 0-7 (pairs 0-3) first; half 1 before pair 4.
            for half in range(2):
                for lc in range(NT):
                    if half == 0 and lc < first_done:
                        continue
                    yield from vt_chain(lc, half)

        def proj_gen(n):
            for m in range(NT):
                acc = fil_ps.tile([128, 512], F32, tag="vac", name="pac")
                for g in range(KG):
                    nc.tensor.matmul(
                        acc, pw_sb[:, g, :, m * 128:(m + 1) * 128],
                        a8[g][:, :, n * 512:(n + 1) * 512],
                        start=(g == 0), stop=(g == KG - 1), perf_mode=DR)
                    yield
                xres = xr_p.tile([128, 512], F32, tag="xres", name="xres")
                nc.vector.tensor_scalar(
                    out=xres, in0=xt[m][:, n * 512:(n + 1) * 512],
                    scalar1=scale_sb[:, m:m + 1], scalar2=bias_sb[:, m:m + 1],
                    op0=mybir.AluOpType.mult, op1=mybir.AluOpType.add)
                o_sb = out_p.tile([128, 512], F32, tag="o_sb", name="o_sb")
                nc.vector.scalar_tensor_tensor(
                    out=o_sb, in0=acc, scalar=pb_sb[:, m:m + 1], in1=xres,
                    op0=mybir.AluOpType.add, op1=mybir.AluOpType.add)
                nc.sync.dma_start(
                    out=out_ap[m * 128:(m + 1) * 128,
                               n * 512:(n + 1) * 512], in_=o_sb)
                yield

        # two v^T chains ahead of window (0,0): its first attention-value
        # matmul (scd 0) needs s-chunks 0,1 for heads 0-1.
        for _ in vt_chain(0, 0):
            pass
        for _ in vt_chain(1, 0):
            pass
        vt = vt_gen(first_done=2)

        def pump(fills, k):
            done = 0
            while done < k and fills:
                try:
                    next(fills[0])
                    done += 1
                except StopIteration:
                    fills.pop(0)

        def window(j, n, fills, pumps_per_sc, finish=()):
            q_j, k_j = qk_tiles[j]
            g_a, s_a = j // 2, j % 2
            ps2 = {par: ps2_p.tile([CH + 1, 512], F32, tag="ps2",
                                   name="ps2") for par in range(2)}
            m1s = {}
            exqs = {}

            def emit_mm1(sc):
                m1 = m1_p.tile([128, 2, 512], F32, tag="m1", name="m1")
                for par in range(2):
                    base = CH * par
                    nc.tensor.matmul(
                        m1[:, par, :],
                        k_j[base:base + CH, sc * 128:(sc + 1) * 128],
                        q_j[base:base + CH, n * 512:(n + 1) * 512],
                    )
                m1s[sc] = m1

            def emit_mm2(scd):
                for par in range(2):
                    nc.tensor.matmul(
                        ps2[par],
                        vT2[scd][:, :, 2 * j + par, :],
                        exqs[scd][:, :, par, :],
                        start=(scd == 0), stop=(scd == KG - 1),
                        perf_mode=DR,
                    )
                del exqs[scd]

            # software-pipelined: mm1 one s-chunk ahead of its exp; the DR
            # attention-value matmul deferred until both its exps retired.
            emit_mm1(0)
            for sc in range(2 * KG):
                scd, sc2 = divmod(sc, 2)
                if sc2 == 0:
                    exqs[scd] = exp_p.tile([128, 2, 2, 512], F8,
                                           tag="ex", name="ex")
                if sc < 2 * KG - 1:
                    emit_mm1(sc + 1)
                if sc2 == 0 and scd >= 1:
                    emit_mm2(scd - 1)
                nc.scalar.activation(
                    out=exqs[scd][:, sc2, :, :], in_=m1s.pop(sc),
                    func=mybir.ActivationFunctionType.Exp,
                    bias=shift_sb, scale=0.125,
                )
                pump(fills, pumps_per_sc)
            # leftover must-finish fillers (next pair's q/k) land here, in
            # the natural PE idle before the last exp-gated mm2.
            for g in finish:
                for _ in g:
                    pass
            emit_mm2(KG - 1)
            # normalization: S copied off the PSUM denominator row (the
            # partition-offset copy is HW-proven), reciprocal on [1,512],
            # gpsimd broadcast, one multiply into a8.
            for par in range(2):
                s_sb = rc_p.tile([1, 512], F32, tag="s_sb", name="s_sb")
                nc.vector.tensor_copy(out=s_sb, in_=ps2[par][CH:CH + 1, :])
                rc = rc_p.tile([1, 512], F32, tag="rc", name="rc")
                nc.vector.reciprocal_approx_fast(out=rc, in_=s_sb)
                rcb = rcb_p.tile([CH, 512], F32, tag="rcb", name="rcb")
                nc.gpsimd.partition_broadcast(rcb, rc, channels=CH)
                nc.vector.tensor_tensor(
                    out=a8[g_a][CH * par:CH * (par + 1), s_a,
                                n * 512:(n + 1) * 512],
                    in0=ps2[par][0:CH, :], in1=rcb,
                    op=mybir.AluOpType.mult)

        proj0 = None
        for n in range(LT):
            for j in range(PAIRS):
                fills = []
                finish = ()
                if n == 0:
                    if j + 1 < PAIRS:
                        qkg = qk_gen_a(j + 1)
                        # window (0,0): vt first -- its own mm2s consume
                        # vT2 chunks that must be EMITTED before them
                        # (in-order PE queue); qk finishes via the
                        # in-window drain.
                        fills = [vt, qkg] if j == 0 else [qkg, vt]
                        finish = (qkg,)
                    else:
                        qb0, qb1 = qk_gen_b(0), qk_gen_b(1)
                        fills = [qb0, qb1, vt]
                        finish = (qb0, qb1)
                else:
                    if proj0 is None:
                        proj0 = proj_gen(0)
                    if j + 2 < PAIRS:
                        qbg = qk_gen_b(j + 2)
                        fills = [qbg, proj0]
                        finish = (qbg,)
                    else:
                        fills = [proj0]
                window(j, n, fills, pumps_per_sc=4 if n == 0 else 3,
                       finish=finish)
        for _ in vt:
            pass
        if proj0 is not None:
            for _ in proj0:
                pass

        # ---- tail: proj n=1 + residual epilogue ----------------------
        for m in range(NT):
            pool = fil_ps if m % 2 == 0 else qkps
            acc = pool.tile([128, 512], F32,
                            tag="vac" if m % 2 == 0 else "qka", name="pta")
            for g in range(KG):
                nc.tensor.matmul(
                    acc, pw_sb[:, g, :, m * 128:(m + 1) * 128],
                    a8[g][:, :, 512:1024],
                    start=(g == 0), stop=(g == KG - 1), perf_mode=DR)
            xres = xr_p.tile([128, 512], F32, tag="xres", name="xres")
            nc.vector.tensor_scalar(
                out=xres, in0=xt[m][:, 512:1024],
                scalar1=scale_sb[:, m:m + 1], scalar2=bias_sb[:, m:m + 1],
                op0=mybir.AluOpType.mult, op1=mybir.AluOpType.add)
            o_sb = out_p.tile([128, 512], F32, tag="o_sb", name="o_sb")
            nc.vector.scalar_tensor_tensor(
                out=o_sb, in0=acc, scalar=pb_sb[:, m:m + 1], in1=xres,
                op0=mybir.AluOpType.add, op1=mybir.AluOpType.add)
            # split the 2MB output tail across two DMA queues (the scalar
            # queue is idle after the last exp)
            eng = nc.sync if m % 2 == 0 else nc.scalar
            eng.dma_start(
                out=out_ap[m * 128:(m + 1) * 128, 512:1024], in_=o_sb)


_CACHED = {}


def build_program(repeats=1):
    key = ("nc", repeats)
    if key in _CACHED:
        return _CACHED[key]
    from contextlib import ExitStack

    nc = bacc.Bacc("TRN2", target_bir_lowering=False, debug=False)
    with tile.TileContext(nc) as tc:
        params = declare_params(nc)
        for rep in range(repeats):
            out_h = None
            if rep > 0:
                out_h = nc.dram_tensor(f"out_scratch{rep}", [C, L], F32)
            with ExitStack() as ctx:
                emit(nc, tc, ctx, params, out_h)
    nc.compile()
    _CACHED[key] = nc
    return nc


def to_f8(a):
    return np.clip(np.asarray(a, np.float32), -240.0, 240.0).astype(
        ml_dtypes.float8_e4m3)


def host_pack(norm_w, norm_b, qkv_w, qkv_b, proj_w, proj_b):
    """Precompute packed weight layouts (all plain numpy)."""
    f = np.float32
    qkv_w = np.asarray(qkv_w, f)
    qkv_b = np.asarray(qkv_b, f)
    proj_w = np.asarray(proj_w, f)
    proj_b = np.asarray(proj_b, f)

    # q/k index packing: pair tile j holds heads 2j (cols 0:64), 2j+1
    idx_q = np.empty(C, np.int64)
    idx_k = np.empty(C, np.int64)
    for j in range(PAIRS):
        for m in range(128):
            h = 2 * j + m // CH
            i = m % CH
            idx_q[j * 128 + m] = 192 * h + i
            idx_k[j * 128 + m] = 192 * h + CH + i
    idx_v = np.empty(C, np.int64)
    for h in range(H):
        idx_v[CH * h:CH * (h + 1)] = 192 * h + 2 * CH + np.arange(CH)

    # DoubleRow packing: [p, ..., g, s, cols], contraction c = 256g+128s+p
    def pack_qk(idx):
        wT = np.ascontiguousarray(qkv_w[idx, :].T)      # [cin, 8*128]
        w = wT.reshape(KG, 2, 128, PAIRS, 128)          # [g, s, p, j, m]
        return to_f8(np.ascontiguousarray(w.transpose(2, 3, 0, 1, 4)))

    q_w8 = pack_qk(idx_q)
    k_w8 = pack_qk(idx_k)

    def pack_cc(wT):                                    # wT: [cin, cols]
        w = wT.reshape(KG, 2, 128, C)                   # [g, s, p, col]
        return to_f8(np.ascontiguousarray(w.transpose(2, 0, 1, 3)))

    v_w8 = pack_cc(np.ascontiguousarray(qkv_w[idx_v, :].T))
    p_w8 = pack_cc(np.ascontiguousarray(proj_w.T))

    q_b = qkv_b[idx_q].reshape(NT, 128).T
    k_b = qkv_b[idx_k].reshape(NT, 128).T
    # v bias passes through softmax exactly -> fold into proj bias
    pbe = proj_b + proj_w @ qkv_b[idx_v]
    proj_beff = pbe.astype(f).reshape(NT, 128).T

    norm_w_c = np.asarray(norm_w, f).reshape(NT, 128).T
    norm_b_c = np.asarray(norm_b, f).reshape(NT, 128).T

    pp = np.arange(128)
    A_grp = (pp[:, None] // 32 == np.arange(4)[None, :]).astype(f)
    A2T = np.ascontiguousarray(A_grp.T)

    # packed consts: [ag(4) | nw(8) | nb(8) | qb(8) | kb(8) | pb(8)]
    cpk = np.ascontiguousarray(np.concatenate(
        [A_grp, norm_w_c, norm_b_c, q_b, k_b, proj_beff],
        axis=1).astype(f))

    return dict(
        q_w8=q_w8, k_w8=k_w8, v_w8=v_w8, p_w8=p_w8,
        cpk=cpk, A2T=A2T,
    )


def kernel(x, norm_w, norm_b, qkv_w, qkv_b, proj_w, proj_b, _trace=False):
    x = np.asarray(x, np.float32)
    shared = host_pack(norm_w, norm_b, qkv_w, qkv_b, proj_w, proj_b)
    nc = build_program()
    in_maps = [dict(shared, x=np.ascontiguousarray(x[i])) for i in range(B)]
    res = run_bass_kernel_spmd(nc, in_maps, list(range(B)), trace=_trace)
    out = np.stack([res.results[i]["out"] for i in range(B)], axis=0)
    if _trace:
        kernel._last_results = res
    return out.astype(np.float32)


# revision 36
# speedup vs baseline: 1.0580x; 1.0580x over previous
"""Trainium2 Bass kernel for nn_AttentionBlock (B=8, C=1024, L=1024, H=16, G=32).

Data-parallel over batch: 8 samples -> 8 NeuronCores, one sample per core.

v2 design notes (vs v1): the kernel is ACT-bound -- the 16.8M softmax exps
cost ~1.03us per 1024-element ACTIVATE at 1.2GHz, ~132us total per core.
Everything else must hide under that stream while keeping the PE busy enough
to hold its 2.4GHz p-state (idle gaps reset the clock ramp to 1.2GHz):

  - x is uploaded as bf16 (2MB not 4MB): the last tile lands ~7us in.
  - GroupNorm per batch-of-4 tiles; the x*scale+bias fp8 applies are split
    into L-halves and alternate ACT/DVE so the n0 halves (which gate the
    first q/k chains) finish early.
  - Attention runs tcn-major (t-chunk outer, head-pair inner).  Window
    (j, n) = 8 scoresT matmul pairs (row-group concurrent bf16), 8 exps
    (the pacing stream), 4 deferred fp8-DR attention-value matmuls, and a
    normalization epilogue (reciprocal on DVE straight from the PSUM
    denominator row, gpsimd partition-broadcast, one multiply into a8).
  - PE filler generators are pumped between exps so the PE never waits:
    next pair's q/k DR projection (chains ordered k-n0, q-n0, k-n1, q-n1
    so the next window's early scores can start after two chains), the
    v^T projection (tcn0 windows), and the output projection for t-chunk 0
    (tcn1 windows).  Only proj n=1 (+epilogue) remains after the last exp.
  - The ACT queue carries ONLY: one table load (natural_log_exp), the two
    GN rsqrts, half the GN applies, and the 128 exps.  All DMAs ride the
    sync/vector/gpsimd queues.
  - PSUM: scores ring 2x[128,2,512] (4 banks) + ps2 2x[65,512] (2) +
    qk accumulator (1) + filler accumulator (1) = 8 banks.

Numerics as v1: fp8e4m3 DoubleRow matmuls for all K=1024 contractions,
bf16 scores, softmax exp(z/8 - 2) without max-subtraction, denominator via
an exact-1.0 column 64 in vT, v bias folded into the proj bias on host.
"""

import numpy as np
import ml_dtypes

import concourse.bass as bass
import concourse.bacc as bacc
import concourse.tile as tile
from concourse import mybir
from concourse.bass_utils import run_bass_kernel_spmd

F32 = mybir.dt.float32
BF16 = mybir.dt.bfloat16
F8 = mybir.dt.float8e4
DR = mybir.MatmulPerfMode.DoubleRow

B, C, L, H = 8, 1024, 1024, 16
GROUPS = 32
CH = C // H          # 64 per-head channels
EPS = 1e-5
NT = C // 128        # 8 channel tiles
LT = L // 512        # 2 free-dim chunks of 512
PAIRS = H // 2       # 8 head pairs
KG = 4               # DoubleRow contraction groups of 256 channels
SHIFT = 2.0          # exp(z/8 - SHIFT); cancels in softmax normalization


def declare_params(nc):
    p = {}
    p["x"] = nc.declare_dram_parameter("x", [C, L], BF16, isOutput=False)
    p["q_w8"] = nc.declare_dram_parameter("q_w8", [128, PAIRS, KG, 2, 128],
                                          F8, isOutput=False)
    p["k_w8"] = nc.declare_dram_parameter("k_w8", [128, PAIRS, KG, 2, 128],
                                          F8, isOutput=False)
    p["v_w8"] = nc.declare_dram_parameter("v_w8", [128, KG, 2, C], F8,
                                          isOutput=False)
    p["p_w8"] = nc.declare_dram_parameter("p_w8", [128, KG, 2, C], F8,
                                          isOutput=False)
    # packed small consts: [ag(4) | nw(8) | nb(8) | qb(8) | kb(8) | pb(8)]
    p["cpk"] = nc.declare_dram_parameter("cpk", [128, 44], F32,
                                         isOutput=False)
    p["A2T"] = nc.declare_dram_parameter("A2T", [4, 128], F32, isOutput=False)
    p["out"] = nc.declare_dram_parameter("out", [C, L], F32, isOutput=True)
    return p


def emit(nc, tc, ctx, params, out_handle=None):
    """Emit one whole per-core computation inside an open TileContext."""
    from contextlib import ExitStack

    x_d = params["x"]
    out_d = params["out"] if out_handle is None else out_handle
    x_ap, out_ap = x_d.ap(), out_d.ap()

    # ---- persistent SBUF pools ---------------------------------------
    consts = ctx.enter_context(tc.tile_pool(name="consts", bufs=1))
    xp = ctx.enter_context(tc.tile_pool(name="xp", bufs=NT))
    xb8_p = ctx.enter_context(tc.tile_pool(name="xb8", bufs=KG))
    wq_p = ctx.enter_context(tc.tile_pool(name="wq", bufs=1))
    vT_p = ctx.enter_context(tc.tile_pool(name="vT", bufs=KG))
    a8_p = ctx.enter_context(tc.tile_pool(name="a8", bufs=KG))
    gn_p = ctx.enter_context(tc.tile_pool(name="gn", bufs=1))
    qk_p = ctx.enter_context(tc.tile_pool(name="qk", bufs=2 * PAIRS))
    exp_p = ctx.enter_context(tc.tile_pool(name="expp", bufs=4))
    rc_p = ctx.enter_context(tc.tile_pool(name="rcp", bufs=2))
    rcb_p = ctx.enter_context(tc.tile_pool(name="rcbp", bufs=2))
    out_p = ctx.enter_context(tc.tile_pool(name="outp", bufs=3))
    xr_p = ctx.enter_context(tc.tile_pool(name="xrp", bufs=3))

    # ---- DMA schedule ------------------------------------------------
    # x rides the wire as bf16 and is cast-DMA'd to f32 SBUF tiles by the
    # gpsimd SWDGE (only engine that can cast) -- halves the critical
    # input traffic while keeping every compute op fp32 (HW-proven).
    # Weight DMAs are throttled behind GroupNorm progress via dummy
    # scalar-queue copies so they don't steal HBM bandwidth from x.
    #   gpsimd: x0..x7 cast DMAs
    #   scalar: qw0/kw0, [after GN b0] vw+qw/kw rest, [after b1] pw
    #   sync:   packed consts (tiny)
    # One manual ACT table load of natural_log_exp_and_others (set 6:
    # exp+ln+identity+copy).  It dominates every activation in the
    # program, so the insert_act_table_loads fixpoint adds no further
    # loads -- without this, walrus assigns Ln and Exp to DIFFERENT sets
    # and the GroupNorm rsqrt thrashes ~2.7us table loads 4 times.
    nc.scalar.add_instruction(mybir.InstLoadActFuncSet(
        name=nc.get_next_instruction_name(), act_func_set_id=6,
        ins=[], outs=[]))

    qw0_sb = wq_p.tile([128, 1, KG, 2, 128], F8, tag="qw0", name="qw0")
    kw0_sb = wq_p.tile([128, 1, KG, 2, 128], F8, tag="kw0", name="kw0")
    nc.scalar.dma_start(out=qw0_sb, in_=params["q_w8"].ap()[:, 0:1])
    nc.scalar.dma_start(out=kw0_sb, in_=params["k_w8"].ap()[:, 0:1])

    onesg = consts.tile([128, 2 * H], F32, tag="onesg", name="onesg")
    nc.vector.memset(onesg, 1.0)
    eps_sb = consts.tile([4, 1], F32, tag="eps", name="eps")
    nc.vector.memset(eps_sb, EPS)
    shift_sb = consts.tile([128, 1], F32, tag="shift", name="shift")
    nc.vector.memset(shift_sb, -SHIFT)

    # packed consts first on sync (one tiny DMA, lands ~1us)
    cpk_sb = consts.tile([128, 44], F32, tag="cpk", name="cpk")
    nc.sync.dma_start(out=cpk_sb, in_=params["cpk"].ap())
    a2_sb = consts.tile([4, 128], F32, tag="a2", name="a2")
    nc.sync.dma_start(out=a2_sb, in_=params["A2T"].ap())
    ag_sb = cpk_sb[:, 0:4]
    nw_sb = cpk_sb[:, 4:12]
    nb_sb = cpk_sb[:, 12:20]
    qb_sb = cpk_sb[:, 20:28]
    kb_sb = cpk_sb[:, 28:36]
    pb_sb = cpk_sb[:, 36:44]

    # x tiles spread over all three DMA rings (each ring sustains only
    # ~160GB/s; one ring alone would take ~13us for 2MB)
    xt = []
    x_eng = [nc.sync, nc.scalar, nc.gpsimd, nc.sync,
             nc.scalar, nc.gpsimd, nc.sync, nc.scalar]
    for t in range(NT):
        tt = xp.tile([128, L], BF16, tag="x_t", name="x_t")
        x_eng[t].dma_start(out=tt, in_=x_ap[t * 128:(t + 1) * 128, :])
        xt.append(tt)

    qwr_sb = wq_p.tile([128, PAIRS - 1, KG, 2, 128], F8, tag="qwr",
                       name="qwr")
    kwr_sb = wq_p.tile([128, PAIRS - 1, KG, 2, 128], F8, tag="kwr",
                       name="kwr")
    vw_sb = wq_p.tile([128, KG, 2, C], F8, tag="vw", name="vw")
    pw_sb = wq_p.tile([128, KG, 2, C], F8, tag="pw", name="pw")

    def qk_w(name, j):
        if j == 0:
            return (qw0_sb if name == "q" else kw0_sb)[:, 0]
        return (qwr_sb if name == "q" else kwr_sb)[:, j - 1]

    # vT2[scp][s, sub, h, 0:64] = v^T for s-chunk scp*2+sub; col 64 == 1.0
    vT2 = []
    for scp in range(KG):
        vt_t = vT_p.tile([128, 2, H, CH + 1], F8, tag="vT_t", name="vT_t")
        nc.vector.tensor_copy(
            out=vt_t[:, :, :, CH:CH + 1],
            in_=onesg.rearrange("p (a g o) -> p a g o", a=2, o=1))
        vT2.append(vt_t)

    xb8 = [xb8_p.tile([128, 2, L], F8, tag="xb8_t", name="xb8_t")
           for _ in range(KG)]
    a8 = [a8_p.tile([128, 2, L], F8, tag="a8_t", name="a8_t")
          for _ in range(KG)]

    # ================= GroupNorm (2 batches of 4 tiles) ===============
    scale_sb = gn_p.tile([128, NT], F32, tag="scale", name="scale")
    bias_sb = gn_p.tile([128, NT], F32, tag="bias", name="bias")
    stats6 = gn_p.tile([128, NT, 2, 6], F32, tag="st6", name="st6")
    mv_all = gn_p.tile([128, NT, 2], F32, tag="mva", name="mva")
    stats = gn_p.tile([128, 2 * NT], F32, tag="stats", name="stats")
    mv16 = gn_p.tile([4, 2 * NT], F32, tag="mv16", name="mv16")
    NB = NT // 2

    qk_tiles = {}

    with ExitStack() as head_ps:
        gn_ps = head_ps.enter_context(
            tc.tile_pool(name="gnps", bufs=2, space=bass.MemorySpace.PSUM))
        qk0_ps = head_ps.enter_context(
            tc.tile_pool(name="qk0ps", bufs=4, space=bass.MemorySpace.PSUM))

        for b in range(2):
            ts0 = b * NB
            sl = slice(ts0, ts0 + NB)
            sl2 = slice(NT + ts0, NT + ts0 + NB)
            for t in range(ts0, ts0 + NB):
                for h2 in range(2):
                    nc.vector.bn_stats(
                        out=stats6[:, t, h2, :],
                        in_=xt[t][:, h2 * 512:(h2 + 1) * 512],
                    )
                nc.vector.bn_aggr(out=mv_all[:, t, :],
                                  in_=stats6[:, t, :, :])
            # stats: [mean_c | e2_c] per channel, e2 = var + mean^2
            nc.vector.tensor_copy(out=stats[:, sl], in_=mv_all[:, sl, 0])
            nc.vector.tensor_tensor(out=stats[:, sl2], in0=mv_all[:, sl, 0],
                                    in1=mv_all[:, sl, 0],
                                    op=mybir.AluOpType.mult)
            nc.vector.tensor_tensor(out=stats[:, sl2], in0=stats[:, sl2],
                                    in1=mv_all[:, sl, 1],
                                    op=mybir.AluOpType.add)

            gps = gn_ps.tile([4, 2 * NB], F32, tag="gps", name="gps")
            nc.tensor.matmul(gps[:, 0:NB], ag_sb, stats[:, sl])
            nc.tensor.matmul(gps[:, NB:], ag_sb, stats[:, sl2])
            inv_n = 1.0 / 32
            nc.vector.tensor_scalar_mul(out=mv16[:, sl], in0=gps[:, 0:NB],
                                        scalar1=inv_n)
            e2 = gn_p.tile([4, NB], F32, tag="e2", name="e2")
            nc.vector.tensor_scalar_mul(out=e2, in0=gps[:, NB:],
                                        scalar1=inv_n)
            m2 = gn_p.tile([4, NB], F32, tag="m2", name="m2")
            nc.vector.tensor_tensor(out=m2, in0=mv16[:, sl],
                                    in1=mv16[:, sl],
                                    op=mybir.AluOpType.mult)
            var = gn_p.tile([4, NB], F32, tag="var", name="var")
            nc.vector.tensor_tensor(out=var, in0=e2, in1=m2,
                                    op=mybir.AluOpType.subtract)
            lnv = gn_p.tile([4, NB], F32, tag="lnv", name="lnv")
            nc.scalar.activation(out=lnv, in_=var,
                                 func=mybir.ActivationFunctionType.Ln,
                                 bias=eps_sb, scale=1.0)
            # istd = exp(-0.5*ln(var+eps)); both in the preloaded set 6
            nc.scalar.activation(out=mv16[:, sl2], in_=lnv,
                                 func=mybir.ActivationFunctionType.Exp,
                                 scale=-0.5)

            bc = gn_ps.tile([128, 2 * NB], F32, tag="bc", name="bc")
            nc.tensor.matmul(bc[:, 0:NB], a2_sb, mv16[:, sl])
            nc.tensor.matmul(bc[:, NB:], a2_sb, mv16[:, sl2])

            nc.vector.tensor_tensor(out=scale_sb[:, sl], in0=nw_sb[:, sl],
                                    in1=bc[:, NB:],
                                    op=mybir.AluOpType.mult)
            tmp = gn_p.tile([128, NB], F32, tag="tmp", name="tmp")
            nc.vector.tensor_tensor(out=tmp, in0=bc[:, 0:NB],
                                    in1=scale_sb[:, sl],
                                    op=mybir.AluOpType.mult)
            nc.vector.tensor_tensor(out=bias_sb[:, sl], in0=nb_sb[:, sl],
                                    in1=tmp, op=mybir.AluOpType.subtract)

            # fp8 applies on DVE, split into L-halves: n0 halves first
            # (they gate the first q/k chains).
            for half in range(2):
                for t in range(ts0, ts0 + NB):
                    nc.vector.tensor_scalar(
                        out=xb8[t // 2][:, t % 2,
                                        half * 512:(half + 1) * 512],
                        in0=xt[t][:, half * 512:(half + 1) * 512],
                        scalar1=scale_sb[:, t:t + 1],
                        scalar2=bias_sb[:, t:t + 1],
                        op0=mybir.AluOpType.mult,
                        op1=mybir.AluOpType.add)

            # weight DMAs throttled behind this batch's GroupNorm result
            # via a REAL WAW dependency: a tiny corner write into each
            # weight tile (sourced from bias_sb) forces the Tile
            # scheduler to start the bulk DMA only after this batch is
            # done -- program order alone is NOT a throttle (the
            # scheduler list-schedules by deps/priority).
            if b == 0:
                targets = [(vw_sb, params["v_w8"]),
                           (qwr_sb, params["q_w8"], (slice(None),
                                                     slice(1, PAIRS))),
                           (kwr_sb, params["k_w8"], (slice(None),
                                                     slice(1, PAIRS)))]
            else:
                targets = [(pw_sb, params["p_w8"])]
            for tgt in targets:
                w_sb, w_d = tgt[0], tgt[1]
                corner = w_sb
                while len(corner.shape) > 2:
                    corner = corner[:, 0]
                nc.vector.tensor_copy(out=corner[0:4, 0:1],
                                      in_=bias_sb[0:4, ts0:ts0 + 1])
                src = w_d.ap()
                if len(tgt) == 3:
                    src = src[tgt[2][0], tgt[2][1]]
                nc.scalar.dma_start(out=w_sb, in_=src)

        # ---- prelude: pair-0 q/k projection (4 parallel PSUM banks) --
        dq0 = qk_p.tile([128, L], BF16, tag="qj", name="qj")
        dk0 = qk_p.tile([128, L], BF16, tag="kj", name="kj")
        qk_tiles[0] = (dq0, dk0)
        chains = (("k", 0), ("q", 0), ("k", 1), ("q", 1))
        accs = {c: qk0_ps.tile([128, 512], F32, tag="qk0a", name="qk0a")
                for c in chains}
        for g in range(KG):
            for name, nn_ in chains:
                nc.tensor.matmul(
                    accs[(name, nn_)], qk_w(name, 0)[:, g],
                    xb8[g][:, :, nn_ * 512:(nn_ + 1) * 512],
                    start=(g == 0), stop=(g == KG - 1), perf_mode=DR)
        for name, nn_ in chains:
            dst = dq0 if name == "q" else dk0
            b_sb = qb_sb if name == "q" else kb_sb
            nc.vector.tensor_scalar_add(
                out=dst[:, nn_ * 512:(nn_ + 1) * 512],
                in0=accs[(name, nn_)], scalar1=b_sb[:, 0:1])

    # ================= attention: tcn-major windows ===================
    with ExitStack() as attn:
        m1_p = attn.enter_context(
            tc.tile_pool(name="m1p", bufs=2, space=bass.MemorySpace.PSUM))
        ps2_p = attn.enter_context(
            tc.tile_pool(name="ps2p", bufs=2, space=bass.MemorySpace.PSUM))
        qkps = attn.enter_context(
            tc.tile_pool(name="qkps", bufs=1, space=bass.MemorySpace.PSUM))
        fil_ps = attn.enter_context(
            tc.tile_pool(name="filps", bufs=1, space=bass.MemorySpace.PSUM))

        def qk_chains(j, chains):
            """q/k DR projection chains for pair j in yield-sized chunks."""
            for name, nn_ in chains:
                acc = qkps.tile([128, 512], F32, tag="qka", name="qka")
                for g in range(KG):
                    nc.tensor.matmul(
                        acc, qk_w(name, j)[:, g],
                        xb8[g][:, :, nn_ * 512:(nn_ + 1) * 512],
                        start=(g == 0), stop=(g == KG - 1), perf_mode=DR)
                    yield
                dst = qk_tiles[j][0 if name == "q" else 1]
                b_sb = qb_sb if name == "q" else kb_sb
                nc.vector.tensor_scalar_add(
                    out=dst[:, nn_ * 512:(nn_ + 1) * 512],
                    in0=acc, scalar1=b_sb[:, j:j + 1])

        def qk_gen_a(j):
            """k-n0, q-n0, k-n1 for pair j (everything its tcn0 window
            touches; q-n1 is deferred to a tcn1 filler)."""
            qk_tiles[j] = (qk_p.tile([128, L], BF16, tag="qj", name="qj"),
                           qk_p.tile([128, L], BF16, tag="kj", name="kj"))
            yield from qk_chains(j, (("k", 0), ("q", 0), ("k", 1)))

        def qk_gen_b(j):
            yield from qk_chains(j, (("q", 1),))

        def vt_chain(lc, half):
            acc = fil_ps.tile([128, 512], F32, tag="vac", name="vac")
            for g in range(KG):
                nc.tensor.matmul(
                    acc, xb8[g][:, :, lc * 128:(lc + 1) * 128],
                    vw_sb[:, g, :, half * 512:(half + 1) * 512],
                    start=(g == 0), stop=(g == KG - 1), perf_mode=DR)
                yield
            nc.vector.tensor_copy(
                out=vT2[lc // 2][:, lc % 2, 8 * half:8 * half + 8, 0:CH],
                in_=acc.rearrange("p (h c) -> p h c", c=CH))

        def vt_gen(first_done):
            # half 0 = heads 0-7 (pairs 0-3) first; half 1 before pair 4.
            for half in range(2):
                for lc in range(NT):
                    if half == 0 and lc < first_done:
                        continue
                    yield from vt_chain(lc, half)

        def proj_gen(n):
            for m in range(NT):
                acc = fil_ps.tile([128, 512], F32, tag="vac", name="pac")
                for g in range(KG):
                    nc.tensor.matmul(
                        acc, pw_sb[:, g, :, m * 128:(m + 1) * 128],
                        a8[g][:, :, n * 512:(n + 1) * 512],
                        start=(g == 0), stop=(g == KG - 1), perf_mode=DR)
                    yield
                xres = xr_p.tile([128, 512], F32, tag="xres", name="xres")
                nc.vector.tensor_scalar(
                    out=xres, in0=xt[m][:, n * 512:(n + 1) * 512],
                    scalar1=scale_sb[:, m:m + 1], scalar2=bias_sb[:, m:m + 1],
                    op0=mybir.AluOpType.mult, op1=mybir.AluOpType.add)
                o_sb = out_p.tile([128, 512], F32, tag="o_sb", name="o_sb")
                nc.vector.scalar_tensor_tensor(
                    out=o_sb, in0=acc, scalar=pb_sb[:, m:m + 1], in1=xres,
                    op0=mybir.AluOpType.add, op1=mybir.AluOpType.add)
                nc.sync.dma_start(
                    out=out_ap[m * 128:(m + 1) * 128,
                               n * 512:(n + 1) * 512], in_=o_sb)
                yield

        # two v^T chains ahead of window (0,0): its first attention-value
        # matmul (scd 0) needs s-chunks 0,1 for heads 0-1.
        for _ in vt_chain(0, 0):
            pass
        for _ in vt_chain(1, 0):
            pass
        vt = vt_gen(first_done=2)

        def pump(fills, k):
            done = 0
            while done < k and fills:
                try:
                    next(fills[0])
                    done += 1
                except StopIteration:
                    fills.pop(0)

        pending = [None]

        def window(j, n, fills, pumps_per_sc, finish=()):
            q_j, k_j = qk_tiles[j]
            g_a, s_a = j // 2, j % 2
            ps2 = {par: ps2_p.tile([CH + 1, 512], F32, tag="ps2",
                                   name="ps2") for par in range(2)}
            m1s = {}
            exqs = {}

            def emit_mm1(sc):
                with tc.high_priority(offset=1 << 20):
                    m1 = m1_p.tile([128, 2, 512], F32, tag="m1", name="m1")
                    for par in range(2):
                        base = CH * par
                        nc.tensor.matmul(
                            m1[:, par, :],
                            k_j[base:base + CH, sc * 128:(sc + 1) * 128],
                            q_j[base:base + CH, n * 512:(n + 1) * 512],
                        )
                    m1s[sc] = m1

            def emit_mm2(scd):
                with tc.high_priority(offset=1 << 20):
                    for par in range(2):
                        nc.tensor.matmul(
                            ps2[par],
                            vT2[scd][:, :, 2 * j + par, :],
                            exqs[scd][:, :, par, :],
                            start=(scd == 0), stop=(scd == KG - 1),
                            perf_mode=DR,
                        )
                del exqs[scd]

            # software-pipelined: mm1 one s-chunk ahead of its exp; the DR
            # attention-value matmul deferred until both its exps retired.
            # The PREVIOUS window's exp-gated close (its last mm2 + norm)
            # is emitted after this window's first mm1, so the next exp
            # is already in flight when the boundary mm2 stalls the PE.
            emit_mm1(0)
            if pending[0] is not None:
                pending[0]()
                pending[0] = None
            for sc in range(2 * KG):
                scd, sc2 = divmod(sc, 2)
                if sc2 == 0:
                    exqs[scd] = exp_p.tile([128, 2, 2, 512], F8,
                                           tag="ex", name="ex")
                if sc < 2 * KG - 1:
                    emit_mm1(sc + 1)
                if sc2 == 0 and scd >= 1:
                    emit_mm2(scd - 1)
                with tc.high_priority(offset=1 << 20):
                    nc.scalar.activation(
                        out=exqs[scd][:, sc2, :, :], in_=m1s.pop(sc),
                        func=mybir.ActivationFunctionType.Exp,
                        bias=shift_sb, scale=0.125,
                    )
                pump(fills, pumps_per_sc)
            # leftover must-finish fillers (next pair's q/k) land here, in
            # the natural PE idle before the last exp-gated mm2.
            for g in finish:
                for _ in g:
                    pass

            def close():
                emit_mm2(KG - 1)
                # normalization: S copied off the PSUM denominator row
                # (the partition-offset copy is HW-proven), reciprocal on
                # [1,512], gpsimd broadcast, one multiply into a8.
                with tc.high_priority(offset=1 << 20):
                    for par in range(2):
                        s_sb = rc_p.tile([1, 512], F32, tag="s_sb",
                                         name="s_sb")
                        nc.vector.tensor_copy(out=s_sb,
                                              in_=ps2[par][CH:CH + 1, :])
                        rc = rc_p.tile([1, 512], F32, tag="rc", name="rc")
                        nc.vector.reciprocal_approx_fast(out=rc, in_=s_sb)
                        rcb = rcb_p.tile([CH, 512], F32, tag="rcb",
                                         name="rcb")
                        nc.gpsimd.partition_broadcast(rcb, rc, channels=CH)
                        nc.vector.tensor_tensor(
                            out=a8[g_a][CH * par:CH * (par + 1), s_a,
                                        n * 512:(n + 1) * 512],
                            in0=ps2[par][0:CH, :], in1=rcb,
                            op=mybir.AluOpType.mult)

            pending[0] = close

        proj0 = None
        for n in range(LT):
            for j in range(PAIRS):
                fills = []
                finish = ()
                if n == 0:
                    if j + 1 < PAIRS:
                        qkg = qk_gen_a(j + 1)
                        # window (0,0): vt first -- its own mm2s consume
                        # vT2 chunks that must be EMITTED before them
                        # (in-order PE queue); qk finishes via the
                        # in-window drain.
                        fills = [vt, qkg] if j == 0 else [qkg, vt]
                        finish = (qkg,)
                    else:
                        qb0, qb1 = qk_gen_b(0), qk_gen_b(1)
                        fills = [qb0, qb1, vt]
                        finish = (qb0, qb1)
                else:
                    if proj0 is None:
                        proj0 = proj_gen(0)
                    if j + 2 < PAIRS:
                        qbg = qk_gen_b(j + 2)
                        fills = [qbg, proj0]
                        finish = (qbg,)
                    else:
                        fills = [proj0]
                window(j, n, fills, pumps_per_sc=4 if n == 0 else 3,
                       finish=finish)
        if pending[0] is not None:
            pending[0]()
            pending[0] = None
        for _ in vt:
            pass
        if proj0 is not None:
            for _ in proj0:
                pass

        # ---- tail: proj n=1 + residual epilogue ----------------------
        for m in range(NT):
            pool = fil_ps if m % 2 == 0 else qkps
            acc = pool.tile([128, 512], F32,
                            tag="vac" if m % 2 == 0 else "qka", name="pta")
            for g in range(KG):
                nc.tensor.matmul(
                    acc, pw_sb[:, g, :, m * 128:(m + 1) * 128],
                    a8[g][:, :, 512:1024],
                    start=(g == 0), stop=(g == KG - 1), perf_mode=DR)
            xres = xr_p.tile([128, 512], F32, tag="xres", name="xres")
            nc.vector.tensor_scalar(
                out=xres, in0=xt[m][:, 512:1024],
                scalar1=scale_sb[:, m:m + 1], scalar2=bias_sb[:, m:m + 1],
                op0=mybir.AluOpType.mult, op1=mybir.AluOpType.add)
            o_sb = out_p.tile([128, 512], F32, tag="o_sb", name="o_sb")
            nc.vector.scalar_tensor_tensor(
                out=o_sb, in0=acc, scalar=pb_sb[:, m:m + 1], in1=xres,
                op0=mybir.AluOpType.add, op1=mybir.AluOpType.add)
            # split the 2MB output tail across two DMA queues (the scalar
            # queue is idle after the last exp)
            eng = nc.sync if m % 2 == 0 else nc.scalar
            eng.dma_start(
                out=out_ap[m * 128:(m + 1) * 128, 512:1024], in_=o_sb)


_CACHED = {}


def build_program(repeats=1):
    key = ("nc", repeats)
    if key in _CACHED:
        return _CACHED[key]
    from contextlib import ExitStack

    nc = bacc.Bacc("TRN2", target_bir_lowering=False, debug=False)
    with tile.TileContext(nc) as tc:
        params = declare_params(nc)
        for rep in range(repeats):
            out_h = None
            if rep > 0:
                out_h = nc.dram_tensor(f"out_scratch{rep}", [C, L], F32)
            with ExitStack() as ctx:
                emit(nc, tc, ctx, params, out_h)
    nc.compile()
    _CACHED[key] = nc
    return nc


def to_f8(a):
    return np.clip(np.asarray(a, np.float32), -240.0, 240.0).astype(
        ml_dtypes.float8_e4m3)


def host_pack(norm_w, norm_b, qkv_w, qkv_b, proj_w, proj_b):
    """Precompute packed weight layouts (all plain numpy)."""
    f = np.float32
    qkv_w = np.asarray(qkv_w, f)
    qkv_b = np.asarray(qkv_b, f)
    proj_w = np.asarray(proj_w, f)
    proj_b = np.asarray(proj_b, f)

    # q/k index packing: pair tile j holds heads 2j (cols 0:64), 2j+1
    idx_q = np.empty(C, np.int64)
    idx_k = np.empty(C, np.int64)
    for j in range(PAIRS):
        for m in range(128):
            h = 2 * j + m // CH
            i = m % CH
            idx_q[j * 128 + m] = 192 * h + i
            idx_k[j * 128 + m] = 192 * h + CH + i
    idx_v = np.empty(C, np.int64)
    for h in range(H):
        idx_v[CH * h:CH * (h + 1)] = 192 * h + 2 * CH + np.arange(CH)

    # DoubleRow packing: [p, ..., g, s, cols], contraction c = 256g+128s+p
    def pack_qk(idx):
        wT = np.ascontiguousarray(qkv_w[idx, :].T)      # [cin, 8*128]
        w = wT.reshape(KG, 2, 128, PAIRS, 128)          # [g, s, p, j, m]
        return to_f8(np.ascontiguousarray(w.transpose(2, 3, 0, 1, 4)))

    q_w8 = pack_qk(idx_q)
    k_w8 = pack_qk(idx_k)

    def pack_cc(wT):                                    # wT: [cin, cols]
        w = wT.reshape(KG, 2, 128, C)                   # [g, s, p, col]
        return to_f8(np.ascontiguousarray(w.transpose(2, 0, 1, 3)))

    v_w8 = pack_cc(np.ascontiguousarray(qkv_w[idx_v, :].T))
    p_w8 = pack_cc(np.ascontiguousarray(proj_w.T))

    q_b = qkv_b[idx_q].reshape(NT, 128).T
    k_b = qkv_b[idx_k].reshape(NT, 128).T
    # v bias passes through softmax exactly -> fold into proj bias
    pbe = proj_b + proj_w @ qkv_b[idx_v]
    proj_beff = pbe.astype(f).reshape(NT, 128).T

    norm_w_c = np.asarray(norm_w, f).reshape(NT, 128).T
    norm_b_c = np.asarray(norm_b, f).reshape(NT, 128).T

    pp = np.arange(128)
    A_grp = (pp[:, None] // 32 == np.arange(4)[None, :]).astype(f)
    A2T = np.ascontiguousarray(A_grp.T)

    # packed consts: [ag(4) | nw(8) | nb(8) | qb(8) | kb(8) | pb(8)]
    cpk = np.ascontiguousarray(np.concatenate(
        [A_grp, norm_w_c, norm_b_c, q_b, k_b, proj_beff],
        axis=1).astype(f))

    return dict(
        q_w8=q_w8, k_w8=k_w8, v_w8=v_w8, p_w8=p_w8,
        cpk=cpk, A2T=A2T,
    )


def kernel(x, norm_w, norm_b, qkv_w, qkv_b, proj_w, proj_b, _trace=False):
    x = np.asarray(x, np.float32).astype(ml_dtypes.bfloat16)
    shared = host_pack(norm_w, norm_b, qkv_w, qkv_b, proj_w, proj_b)
    nc = build_program()
    in_maps = [dict(shared, x=np.ascontiguousarray(x[i])) for i in range(B)]
    res = run_bass_kernel_spmd(nc, in_maps, list(range(B)), trace=_trace)
    out = np.stack([res.results[i]["out"] for i in range(B)], axis=0)
    if _trace:
        kernel._last_results = res
    return out.astype(np.float32)
